# revision 24
# baseline (speedup 1.0000x reference)
"""DeformableStripConv Trainium2 kernel.

Math (exact restatement of the reference):
  off  = conv3x3(x, offset_w) + offset_b              # [6, H, W] per image
  t_h[k] = off[k]   (vertical/fractional-y offsets for the horizontal strip)
  t_v[k] = off[3+k] (horizontal/fractional-x offsets for the vertical strip)
  out_h[o,y,x] = sum_{k,s} hat(t_h[k][y,x] - s) * U_k[o, y+s, x+k-1]
  out_v[o,y,x] = sum_{k,s} hat(t_v[k][y,x] - s) * V_k[o, y+k-1, x+s]
  where U_k = w_h[:,:,0,k] 1x1-conv of x, V_k = w_v[:,:,k,0] 1x1-conv of x,
  hat(u) = max(0, 1-|u|), s in {-2..2} (exact while |t| < 2; true max|t|~1.3),
  out-of-image reads are zero (handled by zero padding).

Implementation per core (one image, batch-parallel over 8 cores):
  - PE: offset conv (channel-major), spatial 128x6 transposes of off,
        per-line 1x1-conv tiles (spatial-major [pix,64] layout), output
        transposes back to channel-major.
  - DVE: hat-weight maps (big fused tensor_scalar ops) + 15 per-line
        scalar_tensor_tensor FMAs (per-partition scalar = per-pixel weight).
  - ACT: PSUM->SBUF drains.
"""

import os
import sys

sys.path.insert(0, "/opt/trn_rl_repo")

_SKIP = set(os.environ.get("KSKIP", "").split(","))

import numpy as np
import ml_dtypes

import concourse.bass as bass
import concourse.bacc as bacc
import concourse.mybir as mybir
from concourse import tile
from concourse.bass_utils import run_bass_kernel_spmd

F32 = mybir.dt.float32
BF16 = mybir.dt.bfloat16
AOP = mybir.AluOpType

B, C, O, H, W, K = 8, 64, 64, 128, 128, 3
PH, PW = H + 6, W + 6  # padded spatial dims, core at [2:130, 2:130] (pad 2 + slack)
NPIX = H * W
SSH = [-2, -1, 0, 1, 2]  # interpolation shifts


def _build_nc(offset_b_host):
    nc = bacc.Bacc()

    x_d = nc.declare_dram_parameter("x", [C, H, W], F32, isOutput=False)
    offw_d = nc.declare_dram_parameter("offw_t", [C, 9, 6], BF16, isOutput=False)
    wh_d = nc.declare_dram_parameter("wh_t", [C, K, O], BF16, isOutput=False)
    wv_d = nc.declare_dram_parameter("wv_t", [C, K, O], BF16, isOutput=False)
    eyef_d = nc.declare_dram_parameter("eye_f32", [128, 128], F32, isOutput=False)
    eyeb_d = nc.declare_dram_parameter("eye_bf16", [128, 128], BF16, isOutput=False)
    out_d = nc.declare_dram_parameter("out", [O, H, W], F32, isOutput=True)
    offset_b = offset_b_host  # host-side floats, folded into map constants

    with tile.TileContext(nc) as tc:
        with (
            tc.tile_pool(name="const", bufs=1) as cpool,
            tc.tile_pool(name="main", bufs=1) as mpool,
            tc.tile_pool(name="acc", bufs=8) as apool,
        ):
            # ---- constants to SBUF ----
            offw = cpool.tile([C, 9, 6], BF16)
            wh = cpool.tile([C, K, O], BF16)
            wv = cpool.tile([C, K, O], BF16)
            eyef = cpool.tile([128, 128], F32)
            eyeb = cpool.tile([128, 128], BF16)
            nc.sync.dma_start(offw[:], offw_d[:])
            nc.sync.dma_start(wh[:], wh_d[:])
            nc.sync.dma_start(wv[:], wv_d[:])
            nc.sync.dma_start(eyef[:], eyef_d[:])
            nc.sync.dma_start(eyeb[:], eyeb_d[:])

            # ---- load x into zero-padded bf16 image ----
            xpad = mpool.tile([C, PH, PW], BF16)
            nc.gpsimd.memset(xpad[:, 0:2, :], 0.0)
            nc.gpsimd.memset(xpad[:, H + 2 : PH, :], 0.0)
            nc.gpsimd.memset(xpad[:, 2 : H + 2, 0:2], 0.0)
            nc.gpsimd.memset(xpad[:, 2 : H + 2, W + 2 : PW], 0.0)
            nc.gpsimd.dma_start(xpad[:, 2 : H + 2, 2 : W + 2], x_d[:])  # f32->bf16

            # ---- offset conv (channel-major): off_cm [6, NPIX] f32 ----
            off_cm = mpool.tile([6, NPIX], F32, tag="vtiles")
            off_row = mpool.tile([128, H, 6], F32, tag="offrow")
            off_col = mpool.tile([128, W, 6], F32, tag="offcol")
            with (
                tc.tile_pool(name="ps_off", bufs=3,
                             space=bass.MemorySpace.PSUM) as ps_off,
                tc.tile_pool(name="ps_tr", bufs=5,
                             space=bass.MemorySpace.PSUM) as ps_tr,
            ):
                RC = 4  # rows per chunk
                for ch in range(H // RC):
                    pt = ps_off.tile([6, RC * W], F32)
                    for t in range(9):
                        ky, kx = t // 3, t % 3
                        rhs = xpad[:, 2 + ch * RC + ky - 1 : 2 + ch * RC + ky - 1 + RC,
                                   2 + kx - 1 : 2 + kx - 1 + W]
                        nc.tensor.matmul(pt[:], offw[:, t, :], rhs,
                                         start=(t == 0), stop=(t == 8))
                    nc.scalar.copy(off_cm[:, ch * RC * W : (ch + 1) * RC * W], pt[:])

                # transpose off to spatial-major (row-chunks now; col-chunks are
                # deferred to just before strip 1 so they overlap strip-0 work):
                # off_row[x, y, j] = off[j, y, x];  off_col[y, x, j] = off[j, y, x]
                ocv = off_cm[:].rearrange("j (y x) -> j y x", y=H)
                for y in range(H if "tr" not in _SKIP else 0):
                    pt = ps_tr.tile([128, 6], F32)
                    nc.tensor.transpose(pt[:], ocv[:, y, :], eyef[0:6, 0:6])
                    nc.scalar.copy(off_row[:, y, :], pt[:])
                for x in range(W if "tr" not in _SKIP else 0):
                    pt = ps_tr.tile([128, 6], F32)
                    nc.tensor.transpose(pt[:], ocv[:, :, x], eyef[0:6, 0:6])
                    nc.scalar.copy(off_col[:, x, :], pt[:])

            # ---- hat weight maps ----
            # maps_h[x, y, k, s] = hat(off[k][y,x] + b[k] - s)
            # maps_v[y, x, k, s] = hat(off[3+k][y,x] + b[3+k] - s)
            # hat(t-s) = max(0, min(1-(t-s), 1+(t-s))); bias b folds into the
            # scalar constants: 1 -/+ (s - b).
            def build_maps(off_sm, jlo):
                maps = mpool.tile([128, 128, K, 5], F32, tag=f"maps{jlo}")
                v1 = mpool.tile([128, 128], F32, tag="v1")
                v2 = mpool.tile([128, 128], F32, tag="v2")
                for k in range(K):
                    b = float(offset_b[jlo + k])
                    t = off_sm[:, :, jlo + k]
                    for si, s in enumerate(SSH):
                        nc.vector.tensor_scalar(out=v1[:], in0=t, scalar1=-1.0,
                                                scalar2=float(1 + s - b),
                                                op0=AOP.mult, op1=AOP.add)
                        nc.vector.tensor_scalar(out=v2[:], in0=t, scalar1=1.0,
                                                scalar2=float(1 - s + b),
                                                op0=AOP.mult, op1=AOP.add)
                        nc.vector.tensor_tensor(out=v1[:], in0=v1[:], in1=v2[:],
                                                op=AOP.min)
                        nc.vector.tensor_scalar(out=maps[:, :, k, si], in0=v1[:],
                                                scalar1=0.0, scalar2=None,
                                                op0=AOP.max)
                return maps

            maps_h = build_maps(off_row, 0)
            maps_v = build_maps(off_col, 3)

            # ---- per-strip: 1x1-conv tiles (spatial-major) + weighted combine ----
            out_h = mpool.tile([O, H, W], BF16, tag="outh")   # [o, y, x]
            out_v = mpool.tile([O, W, H], BF16, tag="outv")   # [o, x, y]

            strip_pools = (
                tc.tile_pool(name="ps_v", bufs=4, space=bass.MemorySpace.PSUM),
                tc.tile_pool(name="ps_o", bufs=4, space=bass.MemorySpace.PSUM),
            )
            ps_v = strip_pools[0].__enter__()
            ps_o = strip_pools[1].__enter__()
            for strip in range(2):
                # V tiles: vt[pix, k, line+2, o]
                vt = mpool.tile([128, K, H + 4, O], BF16, tag="vtiles")
                nc.gpsimd.memset(vt[:, :, 0:2, :], 0.0)
                nc.gpsimd.memset(vt[:, :, H + 2 : H + 4, :], 0.0)
                wmat = wh if strip == 0 else wv
                for ln in range(H if "v" not in _SKIP else 0):
                    pv = ps_v.tile([128, K * O], F32)
                    for k in range(K):
                        if strip == 0:
                            # U_k row ln, cols x+k-1: lhsT = xpad[c, 2+ln, 1+k : 1+k+128]
                            lhsT = xpad[:, 2 + ln, 1 + k : 1 + k + W]
                        else:
                            # V_k col ln, rows y+k-1: lhsT = xpad[c, 1+k : 1+k+128, 2+ln]
                            lhsT = xpad[:, 1 + k : 1 + k + H, 2 + ln]
                        nc.tensor.matmul(pv[:, k * O : (k + 1) * O], lhsT,
                                         wmat[:, k, :], start=True, stop=True)
                    nc.scalar.copy(
                        vt[:, :, ln + 2, :],
                        pv[:].rearrange("p (k o) -> p k o", k=K))

                maps = maps_h if strip == 0 else maps_v
                outt = out_h if strip == 0 else out_v
                for ln in range(H if "stt" not in _SKIP else 0):
                    a0 = apool.tile([128, O], BF16, tag="accA")
                    a1 = apool.tile([128, O], BF16, tag="accB")
                    accs = [a0, a1]
                    n = 0
                    for k in range(K if "stt1" not in _SKIP else 1):
                        for si in range(5 if "stt1" not in _SKIP else 1):
                            src, dst = accs[(n + 1) % 2], accs[n % 2]
                            v_in = vt[:, k, ln + SSH[si] + 2, :]
                            nc.vector.scalar_tensor_tensor(
                                out=dst[:],
                                in0=v_in,
                                scalar=maps[:, ln, k, si : si + 1],
                                in1=(v_in if n == 0 else src[:]),
                                op0=AOP.mult,
                                op1=(AOP.bypass if n == 0 else AOP.add))
                            n += 1
                    fin = accs[(n - 1) % 2]
                    po = ps_o.tile([O, 128], BF16)
                    nc.tensor.transpose(po[:], fin[:], eyeb[:])
                    nc.scalar.copy(outt[:, ln, :], po[:])

            strip_pools[1].__exit__(None, None, None)
            strip_pools[0].__exit__(None, None, None)

            # ---- combine strips and store ----
            ovv = out_v[:].rearrange("o x y -> o y x")
            nc.vector.tensor_tensor(out=out_h[:], in0=out_h[:], in1=ovv, op=AOP.add)
            nc.gpsimd.dma_start(out_d[:], out_h[:])  # bf16 -> f32 cast

    nc.compile()
    return nc


_NC_CACHE = {}


def kernel(x, offset_w, offset_b, w_h, w_v, _trace=False):
    ob = np.asarray(offset_b, np.float32)
    key = ob.tobytes()
    if key not in _NC_CACHE:
        _NC_CACHE[key] = _build_nc([float(v) for v in ob])
    nc = _NC_CACHE[key]

    bf = ml_dtypes.bfloat16
    # offw_t[c, 3*ky+kx, j] = offset_w[j, c, ky, kx]
    offw_t = np.ascontiguousarray(
        np.asarray(offset_w, np.float32).transpose(1, 2, 3, 0).reshape(C, 9, 6)
    ).astype(bf)
    wh_t = np.ascontiguousarray(
        np.asarray(w_h, np.float32)[:, :, 0, :].transpose(1, 2, 0)).astype(bf)
    wv_t = np.ascontiguousarray(
        np.asarray(w_v, np.float32)[:, :, :, 0].transpose(1, 2, 0)).astype(bf)
    eye_f32 = np.eye(128, dtype=np.float32)
    eye_bf16 = np.eye(128, dtype=np.float32).astype(bf)

    xs = np.asarray(x, np.float32)
    in_maps = [
        {
            "x": np.ascontiguousarray(xs[i]),
            "offw_t": offw_t,
            "wh_t": wh_t,
            "wv_t": wv_t,
            "eye_f32": eye_f32,
            "eye_bf16": eye_bf16,
        }
        for i in range(B)
    ]
    res = run_bass_kernel_spmd(nc, in_maps, list(range(B)), trace=_trace,
                               trace_cores=[0] if _trace else None)
    out = np.stack([res.results[i]["out"] for i in range(B)], axis=0)
    if _trace:
        return out.astype(np.float32), res
    return out.astype(np.float32)


if __name__ == "__main__":
    x = np.random.randn(B, C, H, W).astype(np.float32)
    ow = (np.random.randn(6, C, 3, 3) * 0.01).astype(np.float32)
    ob = (np.random.randn(6) * 0.01).astype(np.float32)
    whh = (np.random.randn(O, C, 1, 3) * 0.1).astype(np.float32)
    wvv = (np.random.randn(O, C, 3, 1) * 0.1).astype(np.float32)
    print(kernel(x, ow, ob, whh, wvv).shape)


# revision 28
# speedup vs baseline: 1.0494x; 1.0494x over previous
"""DeformableStripConv Trainium2 kernel.

Math (exact restatement of the reference):
  off  = conv3x3(x, offset_w) + offset_b              # [6, H, W] per image
  t_h[k] = off[k]   (vertical/fractional-y offsets for the horizontal strip)
  t_v[k] = off[3+k] (horizontal/fractional-x offsets for the vertical strip)
  out_h[o,y,x] = sum_{k,s} hat(t_h[k][y,x] - s) * U_k[o, y+s, x+k-1]
  out_v[o,y,x] = sum_{k,s} hat(t_v[k][y,x] - s) * V_k[o, y+k-1, x+s]
  where U_k = w_h[:,:,0,k] 1x1-conv of x, V_k = w_v[:,:,k,0] 1x1-conv of x,
  hat(u) = max(0, 1-|u|), s in {-2..2} (exact while |t| < 2; true max|t|~1.3),
  out-of-image reads are zero (handled by zero padding).

Implementation per core (one image, batch-parallel over 8 cores):
  - PE: offset conv (channel-major), spatial 128x6 transposes of off,
        per-line 1x1-conv tiles (spatial-major [pix,64] layout), output
        transposes back to channel-major.
  - DVE: hat-weight maps (big fused tensor_scalar ops) + 15 per-line
        scalar_tensor_tensor FMAs (per-partition scalar = per-pixel weight).
  - ACT: PSUM->SBUF drains.
"""

import os
import sys

sys.path.insert(0, "/opt/trn_rl_repo")

_SKIP = set(os.environ.get("KSKIP", "").split(","))

import numpy as np
import ml_dtypes

import concourse.bass as bass
import concourse.bacc as bacc
import concourse.mybir as mybir
from concourse import tile
from concourse.bass_utils import run_bass_kernel_spmd

F32 = mybir.dt.float32
BF16 = mybir.dt.bfloat16
AOP = mybir.AluOpType

B, C, O, H, W, K = 8, 64, 64, 128, 128, 3
PH, PW = H + 6, W + 6  # padded spatial dims, core at [2:130, 2:130] (pad 2 + slack)
NPIX = H * W
SSH = [-2, -1, 0, 1, 2]  # interpolation shifts


def _build_nc(offset_b_host):
    nc = bacc.Bacc()

    x_d = nc.declare_dram_parameter("x", [C, H, W], F32, isOutput=False)
    offw_d = nc.declare_dram_parameter("offw_t", [C, 9, 6], BF16, isOutput=False)
    wh_d = nc.declare_dram_parameter("wh_t", [C, K, O], BF16, isOutput=False)
    wv_d = nc.declare_dram_parameter("wv_t", [C, K, O], BF16, isOutput=False)
    eyef_d = nc.declare_dram_parameter("eye_f32", [128, 128], F32, isOutput=False)
    eyeb_d = nc.declare_dram_parameter("eye_bf16", [128, 128], BF16, isOutput=False)
    out_d = nc.declare_dram_parameter("out", [O, H, W], F32, isOutput=True)
    offset_b = offset_b_host  # host-side floats, folded into map constants

    with tile.TileContext(nc) as tc:
        with (
            tc.tile_pool(name="const", bufs=1) as cpool,
            tc.tile_pool(name="main", bufs=1) as mpool,
            tc.tile_pool(name="acc", bufs=24) as apool,
        ):
            # ---- constants to SBUF ----
            offw = cpool.tile([C, 9, 6], BF16)
            wh = cpool.tile([C, K, O], BF16)
            wv = cpool.tile([C, K, O], BF16)
            eyef = cpool.tile([128, 128], F32)
            eyeb = cpool.tile([128, 128], BF16)
            nc.sync.dma_start(offw[:], offw_d[:])
            nc.sync.dma_start(wh[:], wh_d[:])
            nc.sync.dma_start(wv[:], wv_d[:])
            nc.sync.dma_start(eyef[:], eyef_d[:])
            nc.sync.dma_start(eyeb[:], eyeb_d[:])

            # ---- load x into zero-padded bf16 image ----
            xpad = mpool.tile([C, PH, PW], BF16)
            nc.gpsimd.memset(xpad[:, 0:2, :], 0.0)
            nc.gpsimd.memset(xpad[:, H + 2 : PH, :], 0.0)
            nc.gpsimd.memset(xpad[:, 2 : H + 2, 0:2], 0.0)
            nc.gpsimd.memset(xpad[:, 2 : H + 2, W + 2 : PW], 0.0)
            nc.gpsimd.dma_start(xpad[:, 2 : H + 2, 2 : W + 2], x_d[:])  # f32->bf16

            # ---- offset conv (channel-major): off_cm [6, NPIX] f32 ----
            off_cm = mpool.tile([6, NPIX], F32, tag="vtiles")
            off_row = mpool.tile([128, H, 6], F32, tag="offrow")
            off_col = mpool.tile([128, W, 6], F32, tag="offcol")
            with (
                tc.tile_pool(name="ps_off", bufs=3,
                             space=bass.MemorySpace.PSUM) as ps_off,
                tc.tile_pool(name="ps_tr", bufs=5,
                             space=bass.MemorySpace.PSUM) as ps_tr,
            ):
                RC = 4  # rows per chunk
                for ch in range(H // RC):
                    pt = ps_off.tile([6, RC * W], F32)
                    for t in range(9):
                        ky, kx = t // 3, t % 3
                        rhs = xpad[:, 2 + ch * RC + ky - 1 : 2 + ch * RC + ky - 1 + RC,
                                   2 + kx - 1 : 2 + kx - 1 + W]
                        nc.tensor.matmul(pt[:], offw[:, t, :], rhs,
                                         start=(t == 0), stop=(t == 8))
                    nc.scalar.copy(off_cm[:, ch * RC * W : (ch + 1) * RC * W], pt[:])

                # transpose off to spatial-major (row-chunks now; col-chunks are
                # deferred to just before strip 1 so they overlap strip-0 work):
                # off_row[x, y, j] = off[j, y, x];  off_col[y, x, j] = off[j, y, x]
                ocv = off_cm[:].rearrange("j (y x) -> j y x", y=H)
                for y in range(H if "tr" not in _SKIP else 0):
                    pt = ps_tr.tile([128, 6], F32)
                    nc.tensor.transpose(pt[:], ocv[:, y, :], eyef[0:6, 0:6])
                    nc.scalar.copy(off_row[:, y, :], pt[:])
                for x in range(W if "tr" not in _SKIP else 0):
                    pt = ps_tr.tile([128, 6], F32)
                    nc.tensor.transpose(pt[:], ocv[:, :, x], eyef[0:6, 0:6])
                    nc.scalar.copy(off_col[:, x, :], pt[:])

            # ---- hat weight maps ----
            # maps_h[x, y, k, s] = hat(off[k][y,x] + b[k] - s)
            # maps_v[y, x, k, s] = hat(off[3+k][y,x] + b[3+k] - s)
            # hat(t-s) = max(0, min(1-(t-s), 1+(t-s))); bias b folds into the
            # scalar constants: 1 -/+ (s - b).
            def build_maps(off_sm, jlo):
                maps = mpool.tile([128, 128, K, 5], F32, tag=f"maps{jlo}")
                v1 = mpool.tile([128, 128], F32, tag="v1")
                v2 = mpool.tile([128, 128], F32, tag="v2")
                for k in range(K):
                    b = float(offset_b[jlo + k])
                    t = off_sm[:, :, jlo + k]
                    for si, s in enumerate(SSH):
                        nc.vector.tensor_scalar(out=v1[:], in0=t, scalar1=-1.0,
                                                scalar2=float(1 + s - b),
                                                op0=AOP.mult, op1=AOP.add)
                        nc.vector.tensor_scalar(out=v2[:], in0=t, scalar1=1.0,
                                                scalar2=float(1 - s + b),
                                                op0=AOP.mult, op1=AOP.add)
                        nc.vector.tensor_tensor(out=v1[:], in0=v1[:], in1=v2[:],
                                                op=AOP.min)
                        nc.vector.tensor_scalar(out=maps[:, :, k, si], in0=v1[:],
                                                scalar1=0.0, scalar2=None,
                                                op0=AOP.max)
                return maps

            maps_h = build_maps(off_row, 0)
            maps_v = build_maps(off_col, 3)

            # ---- per-strip: 1x1-conv tiles (spatial-major) + weighted combine ----
            out_h = mpool.tile([O, H, W], BF16, tag="outh")   # [o, y, x]
            out_v = mpool.tile([O, W, H], BF16, tag="outv")   # [o, x, y]

            strip_pools = (
                tc.tile_pool(name="ps_v", bufs=5, space=bass.MemorySpace.PSUM),
                tc.tile_pool(name="ps_o", bufs=3, space=bass.MemorySpace.PSUM),
            )
            ps_v = strip_pools[0].__enter__()
            ps_o = strip_pools[1].__enter__()
            for strip in range(2):
                # V tiles: vt[pix, k, line+2, o]
                vt = mpool.tile([128, K, H + 4, O], BF16, tag="vtiles")
                nc.gpsimd.memset(vt[:, :, 0:2, :], 0.0)
                nc.gpsimd.memset(vt[:, :, H + 2 : H + 4, :], 0.0)
                wmat = wh if strip == 0 else wv
                for ln in range(H if "v" not in _SKIP else 0):
                    pv = ps_v.tile([128, K * O], F32)
                    for k in range(K):
                        if strip == 0:
                            # U_k row ln, cols x+k-1: lhsT = xpad[c, 2+ln, 1+k : 1+k+128]
                            lhsT = xpad[:, 2 + ln, 1 + k : 1 + k + W]
                        else:
                            # V_k col ln, rows y+k-1: lhsT = xpad[c, 1+k : 1+k+128, 2+ln]
                            lhsT = xpad[:, 1 + k : 1 + k + H, 2 + ln]
                        nc.tensor.matmul(pv[:, k * O : (k + 1) * O], lhsT,
                                         wmat[:, k, :], start=True, stop=True)
                    nc.scalar.copy(
                        vt[:, :, ln + 2, :],
                        pv[:].rearrange("p (k o) -> p k o", k=K))

                maps = maps_h if strip == 0 else maps_v
                outt = out_h if strip == 0 else out_v
                for ln in range(H if "stt" not in _SKIP else 0):
                    a0 = apool.tile([128, O], BF16, tag="accA")
                    a1 = apool.tile([128, O], BF16, tag="accB")
                    accs = [a0, a1]
                    n = 0
                    for k in range(K if "stt1" not in _SKIP else 1):
                        for si in range(5 if "stt1" not in _SKIP else 1):
                            src, dst = accs[(n + 1) % 2], accs[n % 2]
                            v_in = vt[:, k, ln + SSH[si] + 2, :]
                            nc.vector.scalar_tensor_tensor(
                                out=dst[:],
                                in0=v_in,
                                scalar=maps[:, ln, k, si : si + 1],
                                in1=(v_in if n == 0 else src[:]),
                                op0=AOP.mult,
                                op1=(AOP.bypass if n == 0 else AOP.add))
                            n += 1
                    fin = accs[(n - 1) % 2]
                    po = ps_o.tile([O, 128], BF16)
                    nc.tensor.transpose(po[:], fin[:], eyeb[:])
                    nc.scalar.copy(outt[:, ln, :], po[:])

            strip_pools[1].__exit__(None, None, None)
            strip_pools[0].__exit__(None, None, None)

            # ---- combine strips and store ----
            ovv = out_v[:].rearrange("o x y -> o y x")
            nc.vector.tensor_tensor(out=out_h[:], in0=out_h[:], in1=ovv, op=AOP.add)
            nc.gpsimd.dma_start(out_d[:], out_h[:])  # bf16 -> f32 cast

    nc.compile()
    return nc


_NC_CACHE = {}


def kernel(x, offset_w, offset_b, w_h, w_v, _trace=False):
    ob = np.asarray(offset_b, np.float32)
    key = ob.tobytes()
    if key not in _NC_CACHE:
        _NC_CACHE[key] = _build_nc([float(v) for v in ob])
    nc = _NC_CACHE[key]

    bf = ml_dtypes.bfloat16
    # offw_t[c, 3*ky+kx, j] = offset_w[j, c, ky, kx]
    offw_t = np.ascontiguousarray(
        np.asarray(offset_w, np.float32).transpose(1, 2, 3, 0).reshape(C, 9, 6)
    ).astype(bf)
    wh_t = np.ascontiguousarray(
        np.asarray(w_h, np.float32)[:, :, 0, :].transpose(1, 2, 0)).astype(bf)
    wv_t = np.ascontiguousarray(
        np.asarray(w_v, np.float32)[:, :, :, 0].transpose(1, 2, 0)).astype(bf)
    eye_f32 = np.eye(128, dtype=np.float32)
    eye_bf16 = np.eye(128, dtype=np.float32).astype(bf)

    xs = np.asarray(x, np.float32)
    in_maps = [
        {
            "x": np.ascontiguousarray(xs[i]),
            "offw_t": offw_t,
            "wh_t": wh_t,
            "wv_t": wv_t,
            "eye_f32": eye_f32,
            "eye_bf16": eye_bf16,
        }
        for i in range(B)
    ]
    res = run_bass_kernel_spmd(nc, in_maps, list(range(B)), trace=_trace,
                               trace_cores=[0] if _trace else None)
    out = np.stack([res.results[i]["out"] for i in range(B)], axis=0)
    if _trace:
        return out.astype(np.float32), res
    return out.astype(np.float32)


if __name__ == "__main__":
    x = np.random.randn(B, C, H, W).astype(np.float32)
    ow = (np.random.randn(6, C, 3, 3) * 0.01).astype(np.float32)
    ob = (np.random.randn(6) * 0.01).astype(np.float32)
    whh = (np.random.randn(O, C, 1, 3) * 0.1).astype(np.float32)
    wvv = (np.random.randn(O, C, 3, 1) * 0.1).astype(np.float32)
    print(kernel(x, ow, ob, whh, wvv).shape)


# revision 29
# speedup vs baseline: 1.0497x; 1.0003x over previous
"""DeformableStripConv Trainium2 kernel.

Math (exact restatement of the reference):
  off  = conv3x3(x, offset_w) + offset_b              # [6, H, W] per image
  t_h[k] = off[k]   (vertical/fractional-y offsets for the horizontal strip)
  t_v[k] = off[3+k] (horizontal/fractional-x offsets for the vertical strip)
  out_h[o,y,x] = sum_{k,s} hat(t_h[k][y,x] - s) * U_k[o, y+s, x+k-1]
  out_v[o,y,x] = sum_{k,s} hat(t_v[k][y,x] - s) * V_k[o, y+k-1, x+s]
  where U_k = w_h[:,:,0,k] 1x1-conv of x, V_k = w_v[:,:,k,0] 1x1-conv of x,
  hat(u) = max(0, 1-|u|), s in {-2..2} (exact while |t| < 2; true max|t|~1.3),
  out-of-image reads are zero (handled by zero padding).

Implementation per core (one image, batch-parallel over 8 cores):
  - PE: offset conv (channel-major), spatial 128x6 transposes of off,
        per-line 1x1-conv tiles (spatial-major [pix,64] layout), output
        transposes back to channel-major.
  - DVE: hat-weight maps (big fused tensor_scalar ops) + 15 per-line
        scalar_tensor_tensor FMAs (per-partition scalar = per-pixel weight).
  - ACT: PSUM->SBUF drains.
"""

import os
import sys

sys.path.insert(0, "/opt/trn_rl_repo")

_SKIP = set(os.environ.get("KSKIP", "").split(","))

import numpy as np
import ml_dtypes

import concourse.bass as bass
import concourse.bacc as bacc
import concourse.mybir as mybir
from concourse import tile
from concourse.bass_utils import run_bass_kernel_spmd

F32 = mybir.dt.float32
BF16 = mybir.dt.bfloat16
AOP = mybir.AluOpType

B, C, O, H, W, K = 8, 64, 64, 128, 128, 3
PH, PW = H + 6, W + 6  # padded spatial dims, core at [2:130, 2:130] (pad 2 + slack)
NPIX = H * W
SSH = [-2, -1, 0, 1, 2]  # interpolation shifts


def _build_nc(offset_b_host):
    nc = bacc.Bacc()

    x_d = nc.declare_dram_parameter("x", [C, H, W], F32, isOutput=False)
    offw_d = nc.declare_dram_parameter("offw_t", [C, 9, 6], BF16, isOutput=False)
    wh_d = nc.declare_dram_parameter("wh_t", [C, K, O], BF16, isOutput=False)
    wv_d = nc.declare_dram_parameter("wv_t", [C, K, O], BF16, isOutput=False)
    eyef_d = nc.declare_dram_parameter("eye_f32", [128, 128], F32, isOutput=False)
    eyeb_d = nc.declare_dram_parameter("eye_bf16", [128, 128], BF16, isOutput=False)
    out_d = nc.declare_dram_parameter("out", [O, H, W], F32, isOutput=True)
    offset_b = offset_b_host  # host-side floats, folded into map constants

    with tile.TileContext(nc) as tc:
        with (
            tc.tile_pool(name="const", bufs=1) as cpool,
            tc.tile_pool(name="main", bufs=1) as mpool,
            tc.tile_pool(name="acc", bufs=32) as apool,
        ):
            # ---- constants to SBUF ----
            offw = cpool.tile([C, 9, 6], BF16)
            wh = cpool.tile([C, K, O], BF16)
            wv = cpool.tile([C, K, O], BF16)
            eyef = cpool.tile([128, 128], F32)
            eyeb = cpool.tile([128, 128], BF16)
            nc.sync.dma_start(offw[:], offw_d[:])
            nc.sync.dma_start(wh[:], wh_d[:])
            nc.sync.dma_start(wv[:], wv_d[:])
            nc.sync.dma_start(eyef[:], eyef_d[:])
            nc.sync.dma_start(eyeb[:], eyeb_d[:])

            # ---- load x into zero-padded bf16 image ----
            xpad = mpool.tile([C, PH, PW], BF16)
            nc.gpsimd.memset(xpad[:, 0:2, :], 0.0)
            nc.gpsimd.memset(xpad[:, H + 2 : PH, :], 0.0)
            nc.gpsimd.memset(xpad[:, 2 : H + 2, 0:2], 0.0)
            nc.gpsimd.memset(xpad[:, 2 : H + 2, W + 2 : PW], 0.0)
            nc.gpsimd.dma_start(xpad[:, 2 : H + 2, 2 : W + 2], x_d[:])  # f32->bf16

            # ---- offset conv (channel-major): off_cm [6, NPIX] f32 ----
            off_cm = mpool.tile([6, NPIX], F32, tag="vtiles")
            off_row = mpool.tile([128, H, 6], F32, tag="offrow")
            off_col = mpool.tile([128, W, 6], F32, tag="offcol")
            with (
                tc.tile_pool(name="ps_off", bufs=2,
                             space=bass.MemorySpace.PSUM) as ps_off,
                tc.tile_pool(name="ps_tr", bufs=6,
                             space=bass.MemorySpace.PSUM) as ps_tr,
            ):
                RC = 4  # rows per chunk
                for ch in range(H // RC):
                    pt = ps_off.tile([6, RC * W], F32)
                    for t in range(9):
                        ky, kx = t // 3, t % 3
                        rhs = xpad[:, 2 + ch * RC + ky - 1 : 2 + ch * RC + ky - 1 + RC,
                                   2 + kx - 1 : 2 + kx - 1 + W]
                        nc.tensor.matmul(pt[:], offw[:, t, :], rhs,
                                         start=(t == 0), stop=(t == 8))
                    nc.scalar.copy(off_cm[:, ch * RC * W : (ch + 1) * RC * W], pt[:])

                # transpose off to spatial-major (row-chunks now; col-chunks are
                # deferred to just before strip 1 so they overlap strip-0 work):
                # off_row[x, y, j] = off[j, y, x];  off_col[y, x, j] = off[j, y, x]
                ocv = off_cm[:].rearrange("j (y x) -> j y x", y=H)
                for y in range(H if "tr" not in _SKIP else 0):
                    pt = ps_tr.tile([128, 6], F32)
                    nc.tensor.transpose(pt[:], ocv[:, y, :], eyef[0:6, 0:6])
                    nc.scalar.copy(off_row[:, y, :], pt[:])
                for x in range(W if "tr" not in _SKIP else 0):
                    pt = ps_tr.tile([128, 6], F32)
                    nc.tensor.transpose(pt[:], ocv[:, :, x], eyef[0:6, 0:6])
                    nc.scalar.copy(off_col[:, x, :], pt[:])

            # ---- hat weight maps ----
            # maps_h[x, y, k, s] = hat(off[k][y,x] + b[k] - s)
            # maps_v[y, x, k, s] = hat(off[3+k][y,x] + b[3+k] - s)
            # hat(t-s) = max(0, min(1-(t-s), 1+(t-s))); bias b folds into the
            # scalar constants: 1 -/+ (s - b).
            def build_maps(off_sm, jlo):
                maps = mpool.tile([128, 128, K, 5], F32, tag=f"maps{jlo}")
                v1 = mpool.tile([128, 128], F32, tag="v1")
                v2 = mpool.tile([128, 128], F32, tag="v2")
                for k in range(K):
                    b = float(offset_b[jlo + k])
                    t = off_sm[:, :, jlo + k]
                    for si, s in enumerate(SSH):
                        nc.vector.tensor_scalar(out=v1[:], in0=t, scalar1=-1.0,
                                                scalar2=float(1 + s - b),
                                                op0=AOP.mult, op1=AOP.add)
                        nc.vector.tensor_scalar(out=v2[:], in0=t, scalar1=1.0,
                                                scalar2=float(1 - s + b),
                                                op0=AOP.mult, op1=AOP.add)
                        nc.vector.tensor_tensor(out=v1[:], in0=v1[:], in1=v2[:],
                                                op=AOP.min)
                        nc.vector.tensor_scalar(out=maps[:, :, k, si], in0=v1[:],
                                                scalar1=0.0, scalar2=None,
                                                op0=AOP.max)
                return maps

            maps_h = build_maps(off_row, 0)
            maps_v = build_maps(off_col, 3)

            # ---- per-strip: 1x1-conv tiles (spatial-major) + weighted combine ----
            out_h = mpool.tile([O, H, W], BF16, tag="outh")   # [o, y, x]
            out_v = mpool.tile([O, W, H], BF16, tag="outv")   # [o, x, y]

            strip_pools = (
                tc.tile_pool(name="ps_v", bufs=5, space=bass.MemorySpace.PSUM),
                tc.tile_pool(name="ps_o", bufs=3, space=bass.MemorySpace.PSUM),
            )
            ps_v = strip_pools[0].__enter__()
            ps_o = strip_pools[1].__enter__()
            for strip in range(2):
                # V tiles: vt[pix, k, line+2, o]
                vt = mpool.tile([128, K, H + 4, O], BF16, tag="vtiles")
                nc.gpsimd.memset(vt[:, :, 0:2, :], 0.0)
                nc.gpsimd.memset(vt[:, :, H + 2 : H + 4, :], 0.0)
                wmat = wh if strip == 0 else wv
                for ln in range(H if "v" not in _SKIP else 0):
                    pv = ps_v.tile([128, K * O], F32)
                    for k in range(K):
                        if strip == 0:
                            # U_k row ln, cols x+k-1: lhsT = xpad[c, 2+ln, 1+k : 1+k+128]
                            lhsT = xpad[:, 2 + ln, 1 + k : 1 + k + W]
                        else:
                            # V_k col ln, rows y+k-1: lhsT = xpad[c, 1+k : 1+k+128, 2+ln]
                            lhsT = xpad[:, 1 + k : 1 + k + H, 2 + ln]
                        nc.tensor.matmul(pv[:, k * O : (k + 1) * O], lhsT,
                                         wmat[:, k, :], start=True, stop=True)
                    nc.scalar.copy(
                        vt[:, :, ln + 2, :],
                        pv[:].rearrange("p (k o) -> p k o", k=K))

                maps = maps_h if strip == 0 else maps_v
                outt = out_h if strip == 0 else out_v
                for ln in range(H if "stt" not in _SKIP else 0):
                    a0 = apool.tile([128, O], BF16, tag="accA")
                    a1 = apool.tile([128, O], BF16, tag="accB")
                    accs = [a0, a1]
                    n = 0
                    for k in range(K if "stt1" not in _SKIP else 1):
                        for si in range(5 if "stt1" not in _SKIP else 1):
                            src, dst = accs[(n + 1) % 2], accs[n % 2]
                            v_in = vt[:, k, ln + SSH[si] + 2, :]
                            nc.vector.scalar_tensor_tensor(
                                out=dst[:],
                                in0=v_in,
                                scalar=maps[:, ln, k, si : si + 1],
                                in1=(v_in if n == 0 else src[:]),
                                op0=AOP.mult,
                                op1=(AOP.bypass if n == 0 else AOP.add))
                            n += 1
                    fin = accs[(n - 1) % 2]
                    po = ps_o.tile([O, 128], BF16)
                    nc.tensor.transpose(po[:], fin[:], eyeb[:])
                    nc.scalar.copy(outt[:, ln, :], po[:])

            strip_pools[1].__exit__(None, None, None)
            strip_pools[0].__exit__(None, None, None)

            # ---- combine strips and store ----
            ovv = out_v[:].rearrange("o x y -> o y x")
            nc.vector.tensor_tensor(out=out_h[:], in0=out_h[:], in1=ovv, op=AOP.add)
            nc.gpsimd.dma_start(out_d[:], out_h[:])  # bf16 -> f32 cast

    nc.compile()
    return nc


_NC_CACHE = {}


def kernel(x, offset_w, offset_b, w_h, w_v, _trace=False):
    ob = np.asarray(offset_b, np.float32)
    key = ob.tobytes()
    if key not in _NC_CACHE:
        _NC_CACHE[key] = _build_nc([float(v) for v in ob])
    nc = _NC_CACHE[key]

    bf = ml_dtypes.bfloat16
    # offw_t[c, 3*ky+kx, j] = offset_w[j, c, ky, kx]
    offw_t = np.ascontiguousarray(
        np.asarray(offset_w, np.float32).transpose(1, 2, 3, 0).reshape(C, 9, 6)
    ).astype(bf)
    wh_t = np.ascontiguousarray(
        np.asarray(w_h, np.float32)[:, :, 0, :].transpose(1, 2, 0)).astype(bf)
    wv_t = np.ascontiguousarray(
        np.asarray(w_v, np.float32)[:, :, :, 0].transpose(1, 2, 0)).astype(bf)
    eye_f32 = np.eye(128, dtype=np.float32)
    eye_bf16 = np.eye(128, dtype=np.float32).astype(bf)

    xs = np.asarray(x, np.float32)
    in_maps = [
        {
            "x": np.ascontiguousarray(xs[i]),
            "offw_t": offw_t,
            "wh_t": wh_t,
            "wv_t": wv_t,
            "eye_f32": eye_f32,
            "eye_bf16": eye_bf16,
        }
        for i in range(B)
    ]
    res = run_bass_kernel_spmd(nc, in_maps, list(range(B)), trace=_trace,
                               trace_cores=[0] if _trace else None)
    out = np.stack([res.results[i]["out"] for i in range(B)], axis=0)
    if _trace:
        return out.astype(np.float32), res
    return out.astype(np.float32)


if __name__ == "__main__":
    x = np.random.randn(B, C, H, W).astype(np.float32)
    ow = (np.random.randn(6, C, 3, 3) * 0.01).astype(np.float32)
    ob = (np.random.randn(6) * 0.01).astype(np.float32)
    whh = (np.random.randn(O, C, 1, 3) * 0.1).astype(np.float32)
    wvv = (np.random.randn(O, C, 3, 1) * 0.1).astype(np.float32)
    print(kernel(x, ow, ob, whh, wvv).shape)


# revision 30
# speedup vs baseline: 1.0800x; 1.0288x over previous
"""DeformableStripConv Trainium2 kernel.

Math (exact restatement of the reference):
  off  = conv3x3(x, offset_w) + offset_b              # [6, H, W] per image
  t_h[k] = off[k]   (vertical/fractional-y offsets for the horizontal strip)
  t_v[k] = off[3+k] (horizontal/fractional-x offsets for the vertical strip)
  out_h[o,y,x] = sum_{k,s} hat(t_h[k][y,x] - s) * U_k[o, y+s, x+k-1]
  out_v[o,y,x] = sum_{k,s} hat(t_v[k][y,x] - s) * V_k[o, y+k-1, x+s]
  where U_k = w_h[:,:,0,k] 1x1-conv of x, V_k = w_v[:,:,k,0] 1x1-conv of x,
  hat(u) = max(0, 1-|u|), s in {-2..2} (exact while |t| < 2; true max|t|~1.3),
  out-of-image reads are zero (handled by zero padding).

Implementation per core (one image, batch-parallel over 8 cores):
  - PE: offset conv (channel-major), spatial 128x6 transposes of off,
        per-line 1x1-conv tiles (spatial-major [pix,64] layout), output
        transposes back to channel-major.
  - DVE: hat-weight maps (big fused tensor_scalar ops) + 15 per-line
        scalar_tensor_tensor FMAs (per-partition scalar = per-pixel weight).
  - ACT: PSUM->SBUF drains.
"""

import os
import sys

sys.path.insert(0, "/opt/trn_rl_repo")

_SKIP = set(os.environ.get("KSKIP", "").split(","))

import numpy as np
import ml_dtypes

import concourse.bass as bass
import concourse.bacc as bacc
import concourse.mybir as mybir
from concourse import tile
from concourse.bass_utils import run_bass_kernel_spmd

F32 = mybir.dt.float32
BF16 = mybir.dt.bfloat16
AOP = mybir.AluOpType

B, C, O, H, W, K = 8, 64, 64, 128, 128, 3
PH, PW = H + 6, W + 6  # padded spatial dims, core at [2:130, 2:130] (pad 2 + slack)
NPIX = H * W
SSH = [-2, -1, 0, 1, 2]  # interpolation shifts


def _build_nc(offset_b_host):
    nc = bacc.Bacc()

    x_d = nc.declare_dram_parameter("x", [C, H, W], F32, isOutput=False)
    offw_d = nc.declare_dram_parameter("offw_t", [C, 9, 6], BF16, isOutput=False)
    wh_d = nc.declare_dram_parameter("wh_t", [C, K, O], BF16, isOutput=False)
    wv_d = nc.declare_dram_parameter("wv_t", [C, K, O], BF16, isOutput=False)
    eyef_d = nc.declare_dram_parameter("eye_f32", [128, 128], F32, isOutput=False)
    eyeb_d = nc.declare_dram_parameter("eye_bf16", [128, 128], BF16, isOutput=False)
    out_d = nc.declare_dram_parameter("out", [O, H, W], F32, isOutput=True)
    offset_b = offset_b_host  # host-side floats, folded into map constants

    with tile.TileContext(nc) as tc:
        with (
            tc.tile_pool(name="const", bufs=1) as cpool,
            tc.tile_pool(name="main", bufs=1) as mpool,
            tc.tile_pool(name="acc", bufs=32) as apool,
        ):
            # ---- constants to SBUF ----
            offw = cpool.tile([C, 9, 6], BF16)
            wh = cpool.tile([C, K, O], BF16)
            wv = cpool.tile([C, K, O], BF16)
            eyef = cpool.tile([128, 128], F32)
            eyeb = cpool.tile([128, 128], BF16)
            nc.sync.dma_start(offw[:], offw_d[:])
            nc.sync.dma_start(wh[:], wh_d[:])
            nc.sync.dma_start(wv[:], wv_d[:])
            nc.sync.dma_start(eyef[:], eyef_d[:])
            nc.sync.dma_start(eyeb[:], eyeb_d[:])

            # ---- load x into zero-padded bf16 image ----
            xpad = mpool.tile([C, PH, PW], BF16)
            nc.gpsimd.memset(xpad[:, 0:2, :], 0.0)
            nc.gpsimd.memset(xpad[:, H + 2 : PH, :], 0.0)
            nc.gpsimd.memset(xpad[:, 2 : H + 2, 0:2], 0.0)
            nc.gpsimd.memset(xpad[:, 2 : H + 2, W + 2 : PW], 0.0)
            for b in range(4):
                ys = slice(b * H // 4, (b + 1) * H // 4)
                nc.gpsimd.dma_start(xpad[:, 2 + b * H // 4 : 2 + (b + 1) * H // 4,
                                         2 : W + 2], x_d[:, ys, :])  # f32->bf16

            # ---- offset conv (channel-major): off_cm [6, NPIX] f32 ----
            off_cm = mpool.tile([6, NPIX], F32, tag="vtiles")
            off_row = mpool.tile([128, H, 6], F32, tag="offrow")
            off_col = mpool.tile([128, W, 6], F32, tag="offcol")
            with (
                tc.tile_pool(name="ps_off", bufs=2,
                             space=bass.MemorySpace.PSUM) as ps_off,
                tc.tile_pool(name="ps_tr", bufs=6,
                             space=bass.MemorySpace.PSUM) as ps_tr,
            ):
                RC = 4  # rows per chunk
                for ch in range(H // RC):
                    pt = ps_off.tile([6, RC * W], F32)
                    for t in range(9):
                        ky, kx = t // 3, t % 3
                        rhs = xpad[:, 2 + ch * RC + ky - 1 : 2 + ch * RC + ky - 1 + RC,
                                   2 + kx - 1 : 2 + kx - 1 + W]
                        nc.tensor.matmul(pt[:], offw[:, t, :], rhs,
                                         start=(t == 0), stop=(t == 8))
                    nc.scalar.copy(off_cm[:, ch * RC * W : (ch + 1) * RC * W], pt[:])

                # transpose off to spatial-major (row-chunks now; col-chunks are
                # deferred to just before strip 1 so they overlap strip-0 work):
                # off_row[x, y, j] = off[j, y, x];  off_col[y, x, j] = off[j, y, x]
                ocv = off_cm[:].rearrange("j (y x) -> j y x", y=H)
                for y in range(H if "tr" not in _SKIP else 0):
                    pt = ps_tr.tile([128, 6], F32)
                    nc.tensor.transpose(pt[:], ocv[:, y, :], eyef[0:6, 0:6])
                    nc.scalar.copy(off_row[:, y, :], pt[:])
                for x in range(W if "tr" not in _SKIP else 0):
                    pt = ps_tr.tile([128, 6], F32)
                    nc.tensor.transpose(pt[:], ocv[:, :, x], eyef[0:6, 0:6])
                    nc.scalar.copy(off_col[:, x, :], pt[:])

            # ---- hat weight maps ----
            # maps_h[x, y, k, s] = hat(off[k][y,x] + b[k] - s)
            # maps_v[y, x, k, s] = hat(off[3+k][y,x] + b[3+k] - s)
            # hat(t-s) = max(0, min(1-(t-s), 1+(t-s))); bias b folds into the
            # scalar constants: 1 -/+ (s - b).
            def build_maps(off_sm, jlo):
                maps = mpool.tile([128, 128, K, 5], F32, tag=f"maps{jlo}")
                v1 = mpool.tile([128, 128], F32, tag="v1")
                v2 = mpool.tile([128, 128], F32, tag="v2")
                for k in range(K):
                    b = float(offset_b[jlo + k])
                    t = off_sm[:, :, jlo + k]
                    for si, s in enumerate(SSH):
                        nc.vector.tensor_scalar(out=v1[:], in0=t, scalar1=-1.0,
                                                scalar2=float(1 + s - b),
                                                op0=AOP.mult, op1=AOP.add)
                        nc.vector.tensor_scalar(out=v2[:], in0=t, scalar1=1.0,
                                                scalar2=float(1 - s + b),
                                                op0=AOP.mult, op1=AOP.add)
                        nc.vector.tensor_tensor(out=v1[:], in0=v1[:], in1=v2[:],
                                                op=AOP.min)
                        nc.vector.tensor_scalar(out=maps[:, :, k, si], in0=v1[:],
                                                scalar1=0.0, scalar2=None,
                                                op0=AOP.max)
                return maps

            maps_h = build_maps(off_row, 0)
            maps_v = build_maps(off_col, 3)

            # ---- per-strip: 1x1-conv tiles (spatial-major) + weighted combine ----
            out_h = mpool.tile([O, H, W], BF16, tag="outh")   # [o, y, x]
            out_v = mpool.tile([O, W, H], BF16, tag="outv")   # [o, x, y]

            strip_pools = (
                tc.tile_pool(name="ps_v", bufs=5, space=bass.MemorySpace.PSUM),
                tc.tile_pool(name="ps_o", bufs=3, space=bass.MemorySpace.PSUM),
            )
            ps_v = strip_pools[0].__enter__()
            ps_o = strip_pools[1].__enter__()
            for strip in range(2):
                # V tiles: vt[pix, k, line+2, o]
                vt = mpool.tile([128, K, H + 4, O], BF16, tag="vtiles")
                nc.gpsimd.memset(vt[:, :, 0:2, :], 0.0)
                nc.gpsimd.memset(vt[:, :, H + 2 : H + 4, :], 0.0)
                wmat = wh if strip == 0 else wv
                for ln in range(H if "v" not in _SKIP else 0):
                    pv = ps_v.tile([128, K * O], F32)
                    for k in range(K):
                        if strip == 0:
                            # U_k row ln, cols x+k-1: lhsT = xpad[c, 2+ln, 1+k : 1+k+128]
                            lhsT = xpad[:, 2 + ln, 1 + k : 1 + k + W]
                        else:
                            # V_k col ln, rows y+k-1: lhsT = xpad[c, 1+k : 1+k+128, 2+ln]
                            lhsT = xpad[:, 1 + k : 1 + k + H, 2 + ln]
                        nc.tensor.matmul(pv[:, k * O : (k + 1) * O], lhsT,
                                         wmat[:, k, :], start=True, stop=True)
                    nc.scalar.copy(
                        vt[:, :, ln + 2, :],
                        pv[:].rearrange("p (k o) -> p k o", k=K))

                maps = maps_h if strip == 0 else maps_v
                outt = out_h if strip == 0 else out_v
                for ln in range(H if "stt" not in _SKIP else 0):
                    a0 = apool.tile([128, O], BF16, tag="accA")
                    a1 = apool.tile([128, O], BF16, tag="accB")
                    accs = [a0, a1]
                    n = 0
                    for k in range(K if "stt1" not in _SKIP else 1):
                        for si in range(5 if "stt1" not in _SKIP else 1):
                            src, dst = accs[(n + 1) % 2], accs[n % 2]
                            v_in = vt[:, k, ln + SSH[si] + 2, :]
                            nc.vector.scalar_tensor_tensor(
                                out=dst[:],
                                in0=v_in,
                                scalar=maps[:, ln, k, si : si + 1],
                                in1=(v_in if n == 0 else src[:]),
                                op0=AOP.mult,
                                op1=(AOP.bypass if n == 0 else AOP.add))
                            n += 1
                    fin = accs[(n - 1) % 2]
                    po = ps_o.tile([O, 128], BF16)
                    nc.tensor.transpose(po[:], fin[:], eyeb[:])
                    nc.scalar.copy(outt[:, ln, :], po[:])

            strip_pools[1].__exit__(None, None, None)
            strip_pools[0].__exit__(None, None, None)

            # ---- combine strips and store ----
            ovv = out_v[:].rearrange("o x y -> o y x")
            NB = 4
            for b in range(NB):
                ys = slice(b * H // NB, (b + 1) * H // NB)
                nc.vector.tensor_tensor(out=out_h[:, ys, :], in0=out_h[:, ys, :],
                                        in1=ovv[:, ys, :], op=AOP.add)
                nc.gpsimd.dma_start(out_d[:, ys, :], out_h[:, ys, :])  # bf16->f32

    nc.compile()
    return nc


_NC_CACHE = {}


def kernel(x, offset_w, offset_b, w_h, w_v, _trace=False):
    ob = np.asarray(offset_b, np.float32)
    key = ob.tobytes()
    if key not in _NC_CACHE:
        _NC_CACHE[key] = _build_nc([float(v) for v in ob])
    nc = _NC_CACHE[key]

    bf = ml_dtypes.bfloat16
    # offw_t[c, 3*ky+kx, j] = offset_w[j, c, ky, kx]
    offw_t = np.ascontiguousarray(
        np.asarray(offset_w, np.float32).transpose(1, 2, 3, 0).reshape(C, 9, 6)
    ).astype(bf)
    wh_t = np.ascontiguousarray(
        np.asarray(w_h, np.float32)[:, :, 0, :].transpose(1, 2, 0)).astype(bf)
    wv_t = np.ascontiguousarray(
        np.asarray(w_v, np.float32)[:, :, :, 0].transpose(1, 2, 0)).astype(bf)
    eye_f32 = np.eye(128, dtype=np.float32)
    eye_bf16 = np.eye(128, dtype=np.float32).astype(bf)

    xs = np.asarray(x, np.float32)
    in_maps = [
        {
            "x": np.ascontiguousarray(xs[i]),
            "offw_t": offw_t,
            "wh_t": wh_t,
            "wv_t": wv_t,
            "eye_f32": eye_f32,
            "eye_bf16": eye_bf16,
        }
        for i in range(B)
    ]
    res = run_bass_kernel_spmd(nc, in_maps, list(range(B)), trace=_trace,
                               trace_cores=[0] if _trace else None)
    out = np.stack([res.results[i]["out"] for i in range(B)], axis=0)
    if _trace:
        return out.astype(np.float32), res
    return out.astype(np.float32)


if __name__ == "__main__":
    x = np.random.randn(B, C, H, W).astype(np.float32)
    ow = (np.random.randn(6, C, 3, 3) * 0.01).astype(np.float32)
    ob = (np.random.randn(6) * 0.01).astype(np.float32)
    whh = (np.random.randn(O, C, 1, 3) * 0.1).astype(np.float32)
    wvv = (np.random.randn(O, C, 3, 1) * 0.1).astype(np.float32)
    print(kernel(x, ow, ob, whh, wvv).shape)
